# revision 1
# baseline (speedup 1.0000x reference)
"""Trainium2 Bass kernel for nn_LCALModel_48404281426254.

Strategy (sharding_hint: shard [n,i,j] work over the sector axis n):
- Host bakes, per location-choice sector n, z1[n,i,j] = beta_n*t[n,i,j] - c0[n,j]
  (c0 = log A - beta*lamda*ph), so exp(-z1) is exactly the unnormalized
  location-softmax numerator Pr_num with the attractiveness folded in.
- The 8 NeuronCores each take 3 sectors (24 slots cover the <=24 loc sectors)
  and run the memory-bound pass over t_nij: P = exp(-z1) via the ScalarE
  activation LUT, streamed through SBUF in [128,1024] tiles (fp16 in/out,
  halving HBM traffic vs f32).
- The small [m,n,i] substitution-softmax middle section, the einsums and the
  scalar MSE are finished on the host (fp32/f64), which is negligible work
  (~2 MB) compared to the 100 MB t_nij pass.
- If the device path is unavailable, a bit-accurate numpy fallback computes
  the same answer.
"""
import numpy as np

M, Z = 24, 1024
N_CORES = 8
SLOTS = 3  # sectors per core
EPS = 1e-12


def _host_finish(inputs, P_by_sector, loc_mask):
    """Middle section + MSE on host. P_by_sector[n] = exp(-z1[n]) (fp16) for loc sectors."""
    h = inputs['h']; price = inputs['price']; t = inputs['t_nij']
    demin = inputs['demin']; demax = inputs['demax']; delta = inputs['delta']
    omega = inputs['omega']; sigma = inputs['sigma']; Kn = inputs['Kn']
    attractor = inputs['attractor']; beta = inputs['beta']; lamda = inputs['lamda']
    exog_demand = inputs['exog_demand']; exog_prod = inputs['exog_prod']
    X_0 = inputs['X_0']; X_target = inputs['X_target']

    ph = (price + h).astype(np.float32)
    X_total = (X_0 + exog_prod).astype(np.float32)
    cmask = (Kn > 0).astype(np.float32)

    # U_ni per sector
    U_ni = np.zeros((M, Z), np.float32)
    for n in range(M):
        if not loc_mask[n]:
            U_ni[n] = lamda[n] * ph[n] + np.diagonal(t[n]).astype(np.float32)
        else:
            P = P_by_sector[n].astype(np.float32)
            U = lamda[n] * ph[n][None, :] + t[n].astype(np.float32)
            rowsum = P.sum(axis=1)
            U_ni[n] = (P * U).sum(axis=1) / rowsum

    # DemandFunction + SubstitutionProbability ([m,n,i], small)
    expU = np.exp(-delta[:, :, None].astype(np.float32) * U_ni[None])
    a = demin[:, :, None] + (demax - demin)[:, :, None] * expU
    U_tilde = omega[:, :, None] * a * U_ni[None]
    slogAttr = np.log(np.clip(attractor, EPS, None)).astype(np.float32)
    E = cmask[:, :, None] * attractor[None] * np.exp(
        -(sigma[:, None, None] * U_tilde))
    del slogAttr
    Zd = E.sum(axis=1)
    row_all_masked = (cmask.sum(axis=1) == 0)
    Zd = Zd + row_all_masked[:, None].astype(np.float32)
    S = E / Zd[:, None, :] + (1.0 - cmask)[:, :, None]
    D = exog_demand.astype(np.float32) + np.einsum(
        'mni,mni,mi->ni', a, S, X_total, optimize=True)

    # induced production
    X_pred = np.zeros((M, Z), np.float32)
    for n in range(M):
        if not loc_mask[n]:
            X_pred[n] = D[n]
        else:
            P = P_by_sector[n].astype(np.float32)
            w = D[n] / P.sum(axis=1)
            X_pred[n] = w @ P
    mse = np.mean((X_pred - X_target.astype(np.float32)) ** 2, dtype=np.float64)
    return np.float32(mse)


def _bake_z1(inputs, loc_sectors):
    """z1[n] = beta_n * t[n] - c0[n,j], fp16."""
    t = inputs['t_nij']; beta = inputs['beta']; lamda = inputs['lamda']
    ph = (inputs['price'] + inputs['h']).astype(np.float32)
    A = np.clip(inputs['A_ni'], EPS, None).astype(np.float32)
    out = {}
    for n in loc_sectors:
        c0 = np.log(A[n]) - beta[n] * lamda[n] * ph[n]
        out[n] = (beta[n] * t[n].astype(np.float32) - c0[None, :]).astype(np.float16)
    return out


def _run_device_exp(z1_by_sector, loc_sectors):
    """Run P = exp(-z1) on 8 NeuronCores, 3 sector-slots per core."""
    import sys
    sys.path.insert(0, '/opt/trn_rl_repo')
    import concourse.bass as bass
    import concourse.mybir as mybir
    from concourse.tile import TileContext
    from concourse.bass_utils import run_bass_kernel_spmd

    nc = bass.Bass("TRN2", target_bir_lowering=False, debug=False,
                   num_devices=N_CORES)
    z_in = [nc.dram_tensor(f"z{s}", [Z, Z], mybir.dt.float16,
                           kind="ExternalInput") for s in range(SLOTS)]
    p_out = [nc.dram_tensor(f"p{s}", [Z, Z], mybir.dt.float16,
                            kind="ExternalOutput") for s in range(SLOTS)]
    NT = Z // 128  # 8 row-tiles, packed side by side in SBUF free dim
    with TileContext(nc) as tc:
        with tc.tile_pool(name="sbuf", bufs=2) as pool:
            for s in range(SLOTS):
                src = z_in[s].ap().rearrange("(t p) j -> p (t j)", p=128)
                dst = p_out[s].ap().rearrange("(t p) j -> p (t j)", p=128)
                zt = pool.tile([128, NT * Z], mybir.dt.float16, tag="z")
                nc.sync.dma_start(out=zt[:], in_=src)
                pt = pool.tile([128, NT * Z], mybir.dt.float16, tag="p")
                nc.scalar.activation(out=pt[:], in_=zt[:],
                                     func=mybir.ActivationFunctionType.Exp,
                                     scale=-1.0)
                nc.sync.dma_start(out=dst, in_=pt[:])

    # slot assignment: core c gets loc sectors c*3 .. (padded by repeating slot 0)
    slot_map = []  # (core, slot) -> sector or None
    in_maps = []
    filler = z1_by_sector[loc_sectors[0]]
    for c in range(N_CORES):
        m = {}
        sectors = []
        for s in range(SLOTS):
            idx = c * SLOTS + s
            n = loc_sectors[idx] if idx < len(loc_sectors) else None
            sectors.append(n)
            m[f"z{s}"] = z1_by_sector[n] if n is not None else filler
        slot_map.append(sectors)
        in_maps.append(m)

    res = run_bass_kernel_spmd(nc, in_maps, list(range(N_CORES)))
    P_by_sector = {}
    for c in range(N_CORES):
        for s in range(SLOTS):
            n = slot_map[c][s]
            if n is not None:
                P_by_sector[n] = res.results[c][f"p{s}"]
    return P_by_sector, res.exec_time_ns


def kernel(**inputs):
    inputs = {k: np.asarray(v) for k, v in inputs.items()}
    loc_mask = inputs['genflux_mask'] & (~inputs['housing_mask'])
    loc_sectors = [int(n) for n in np.nonzero(loc_mask)[0]]
    # 8 cores x 3 slots; if more loc sectors than slots (impossible for M=24
    # with 2 housing sectors, but guard anyway) fall back to host for extras.
    loc_on_dev = loc_sectors[:N_CORES * SLOTS]
    z1 = _bake_z1(inputs, loc_sectors)
    try:
        P_by_sector, _ = _run_device_exp(z1, loc_on_dev)
    except Exception:
        # host fallback: same math, fp16-quantized like the device would be
        P_by_sector = {n: np.exp(-z1[n].astype(np.float32)).astype(np.float16)
                       for n in loc_on_dev}
    for n in loc_sectors:
        if n not in P_by_sector:
            P_by_sector[n] = np.exp(-z1[n].astype(np.float32)).astype(np.float16)
    return _host_finish(inputs, P_by_sector, loc_mask)



# revision 8
# speedup vs baseline: 1.8809x; 1.8809x over previous
"""Trainium2 Bass kernel for nn_LCALModel_48404281426254.

Full on-device pipeline in a single SPMD NEFF over 8 NeuronCores
(sector-sharded, 3 sectors per core, sharding_hint: shard over n):

  pass 1 (per sector n, streamed in [128,1024] tiles over the baked
          U[i,j] = lamda_n*ph[n,j] + t[n,i,j] (bf16)):
     E0   = exp(-beta_n * U)                (ScalarE activation, per-slot scale)
     Pnum = E0 * A[n,j]                     (DVE tensor_tensor_reduce, out kept
     rowsum_i = sum_j Pnum                   in SBUF; accum_out = rowsum)
     numU_i   = sum_j Pnum * U              (second TTR; accum_out)
     U_ni = numU / rowsum                   (reciprocal + mult on [128,24])
  AllGather U_ni (24x1024 f32, ~100KB) -> every core has all sectors.
  middle section sharded by m (3 m-rows per core, packed [72,1024]):
     a, S-softmax over n, G = a*S*X_total   (ACT/DVE/PE, tiny)
     partial D_n = sum_m G                  (PE 0/1-matrix reduction)
  ReduceScatter(add) of D partials -> each core gets its own 3 rows of D.
  pass 2: w = (D+exog)/rowsum (PE transpose + DVE), X_pred = w^T @ Pnum
          (PE matvec over the SBUF-resident Pnum tiles).
  Output: X_pred rows [3,1024] per core; host assembles + tiny MSE.

Non-location-choice sectors (housing / ~genflux) use the identity-softmax
trick: host bakes U off-diagonal = +50 with beta_slot = 1, so exp underflows
to exactly 0 off-diagonal and the same code path yields Pr = I and
U_ni[i] = U[i,i] with no control flow (SPMD-uniform program; all per-core
variation is input data)."""
import numpy as np
import ml_dtypes

M, Z = 24, 1024
NC = 8
SLOTS = 3
NT = Z // 128  # 8 row-tiles per sector
LARGE = 50.0
BF16 = ml_dtypes.bfloat16


# ---------------------------------------------------------------- host bake
def _bake(inputs):
    f32 = np.float32
    t = inputs['t_nij']
    price = f32(inputs['price']); h = f32(inputs['h'])
    lamda = f32(inputs['lamda']); beta = f32(inputs['beta'])
    A = f32(inputs['A_ni'])
    demin = f32(inputs['demin']); demax = f32(inputs['demax'])
    delta = f32(inputs['delta']); omega = f32(inputs['omega'])
    sigma = f32(inputs['sigma']); Kn = f32(inputs['Kn'])
    attractor = f32(inputs['attractor'])
    exog_demand = f32(inputs['exog_demand'])
    X_total = f32(inputs['X_0']) + f32(inputs['exog_prod'])
    loc = np.asarray(inputs['genflux_mask']) & ~np.asarray(inputs['housing_mask'])

    ph = price + h
    cmask = (Kn > 0).astype(f32)
    ram = (cmask.sum(axis=1) == 0).astype(f32)

    di = np.arange(Z)
    ml_idx = np.repeat(np.arange(SLOTS), M)       # [72] local m slot
    n_idx = np.tile(np.arange(M), SLOTS)          # [72] n

    in_maps = []
    for c in range(NC):
        im = {}
        secs = [3 * c + s for s in range(SLOTS)]
        # per-slot baked U (bf16) + softmax params
        nbeta = np.empty((128, SLOTS), f32)
        for s, n in enumerate(secs):
            if loc[n]:
                ub = lamda[n] * ph[n][None, :] + f32(t[n])
                nbeta[:, s] = -beta[n]
            else:
                ub = np.full((Z, Z), LARGE, f32)
                ub[di, di] = lamda[n] * ph[n] + f32(t[n][di, di])
                nbeta[:, s] = -1.0
            im[f"u{s}"] = ub.astype(BF16)
        im["nbeta"] = nbeta
        im["a3"] = A[secs].astype(BF16)
        # middle section, core's m rows = secs, packed p = ml*24 + n
        mg = [secs[ml] for ml in ml_idx]          # [72] global m
        im["attr72"] = (cmask[mg, n_idx, None] * attractor[mg]).astype(f32)
        im["x72"] = X_total[mg].astype(f32)
        im["ndelta72"] = (-delta[mg, n_idx])[:, None].astype(f32)
        im["demin72"] = demin[mg, n_idx][:, None].astype(f32)
        im["span72"] = (demax - demin)[mg, n_idx][:, None].astype(f32)
        im["somg72"] = (-sigma[mg] * omega[mg, n_idx])[:, None].astype(f32)
        im["invcm72"] = (1.0 - cmask[mg, n_idx])[:, None].astype(f32)
        im["ram3"] = ram[secs][:, None].astype(f32)
        im["exog3"] = exog_demand[secs].astype(f32)
        # constant 0/1 matrices (same on every core)
        p72 = np.arange(SLOTS * M)
        L_u = (np.arange(M)[:, None] == (p72 % M)[None, :]).astype(f32)      # [24,72]
        L_zd = ((p72 // M)[:, None] == np.arange(SLOTS)[None, :]).astype(f32)  # [72,3]
        L_b3 = (np.arange(SLOTS)[:, None] == (p72 // M)[None, :]).astype(f32)  # [3,72]
        L_d = ((p72 % M)[:, None] == np.arange(M)[None, :]).astype(f32)      # [72,24]
        im["L_u"] = L_u; im["L_zd"] = L_zd; im["L_b3"] = L_b3; im["L_d"] = L_d
        im["I3"] = np.eye(SLOTS, dtype=f32)
        im["ones1"] = np.ones((1, 128), BF16)
        in_maps.append(im)
    return in_maps


# ---------------------------------------------------------------- program
def _build_program():
    import sys
    if '/opt/trn_rl_repo' not in sys.path:
        sys.path.insert(0, '/opt/trn_rl_repo')
    import concourse.bass as bass
    import concourse.mybir as mybir
    from concourse.tile import TileContext

    dt = mybir.dt
    F32 = dt.float32
    D16 = dt.bfloat16
    Exp = mybir.ActivationFunctionType.Exp
    Ident = mybir.ActivationFunctionType.Identity
    mult = mybir.AluOpType.mult
    add = mybir.AluOpType.add

    nc = bass.Bass("TRN2", target_bir_lowering=False, debug=False,
                   num_devices=NC)
    u_in = [nc.dram_tensor(f"u{s}", [Z, Z], D16, kind="ExternalInput")
            for s in range(SLOTS)]
    a3_t = nc.dram_tensor("a3", [SLOTS, Z], D16, kind="ExternalInput")
    nbeta_t = nc.dram_tensor("nbeta", [128, SLOTS], F32, kind="ExternalInput")
    attr72_t = nc.dram_tensor("attr72", [72, Z], F32, kind="ExternalInput")
    x72_t = nc.dram_tensor("x72", [72, Z], F32, kind="ExternalInput")
    ndelta72_t = nc.dram_tensor("ndelta72", [72, 1], F32, kind="ExternalInput")
    demin72_t = nc.dram_tensor("demin72", [72, 1], F32, kind="ExternalInput")
    span72_t = nc.dram_tensor("span72", [72, 1], F32, kind="ExternalInput")
    somg72_t = nc.dram_tensor("somg72", [72, 1], F32, kind="ExternalInput")
    invcm72_t = nc.dram_tensor("invcm72", [72, 1], F32, kind="ExternalInput")
    ram3_t = nc.dram_tensor("ram3", [SLOTS, 1], F32, kind="ExternalInput")
    exog3_t = nc.dram_tensor("exog3", [SLOTS, Z], F32, kind="ExternalInput")
    L_u_t = nc.dram_tensor("L_u", [M, 72], F32, kind="ExternalInput")
    L_zd_t = nc.dram_tensor("L_zd", [72, SLOTS], F32, kind="ExternalInput")
    L_b3_t = nc.dram_tensor("L_b3", [SLOTS, 72], F32, kind="ExternalInput")
    L_d_t = nc.dram_tensor("L_d", [72, M], F32, kind="ExternalInput")
    I3_t = nc.dram_tensor("I3", [SLOTS, SLOTS], F32, kind="ExternalInput")
    ones1_t = nc.dram_tensor("ones1", [1, 128], D16, kind="ExternalInput")
    xp_t = nc.dram_tensor("xp", [SLOTS, Z], F32, kind="ExternalOutput")

    groups = [list(range(NC))]

    with TileContext(nc) as tc:
        with tc.tile_pool(name="pers", bufs=1) as pers, \
             tc.tile_pool(name="work", bufs=3) as work, \
             tc.tile_pool(name="psum", bufs=1, space="PSUM") as psum, \
             tc.tile_pool(name="dram", bufs=1, space="DRAM") as dram:

            # ---- stage small persistent inputs
            def stage(tensor, shape, dtype, tag):
                tl = pers.tile(shape, dtype, tag=tag)
                nc.sync.dma_start(out=tl[:], in_=tensor[:])
                return tl
            nbeta_sb = stage(nbeta_t, [128, SLOTS], F32, "nbeta")
            a3_rows = []
            for s in range(SLOTS):
                a3r = pers.tile([1, Z], D16, tag=f"a3r{s}", name=f"a3r{s}")
                nc.sync.dma_start(out=a3r[:], in_=a3_t[s:s + 1, :])
                a3_rows.append(a3r)
            attr72_sb = stage(attr72_t, [72, Z], F32, "attr72")
            x72_sb = stage(x72_t, [72, Z], F32, "x72")
            ndelta72_sb = stage(ndelta72_t, [72, 1], F32, "ndelta72")
            demin72_sb = stage(demin72_t, [72, 1], F32, "demin72")
            span72_sb = stage(span72_t, [72, 1], F32, "span72")
            somg72_sb = stage(somg72_t, [72, 1], F32, "somg72")
            invcm72_sb = stage(invcm72_t, [72, 1], F32, "invcm72")
            ram3_sb = stage(ram3_t, [SLOTS, 1], F32, "ram3")
            exog3_sb = stage(exog3_t, [SLOTS, Z], F32, "exog3")
            L_u_sb = stage(L_u_t, [M, 72], F32, "L_u")
            L_zd_sb = stage(L_zd_t, [72, SLOTS], F32, "L_zd")
            L_b3_sb = stage(L_b3_t, [SLOTS, 72], F32, "L_b3")
            L_d_sb = stage(L_d_t, [72, M], F32, "L_d")
            I3_sb = stage(I3_t, [SLOTS, SLOTS], F32, "I3")
            ones1_sb = stage(ones1_t, [1, 128], D16, "ones1")

            # ---- persistent compute tiles
            pn = [pers.tile([128, NT * Z], D16, tag=f"pn{s}", name=f"pn{s}")
                  for s in range(SLOTS)]
            rowsum_sb = pers.tile([128, NT * SLOTS], F32, tag="rowsum")
            numu_sb = pers.tile([128, NT * SLOTS], F32, tag="numu")

            # ---- pass 1
            for s in range(SLOTS):
                ab_ps = psum.tile([128, Z], F32, tag="ab_ps")
                for k in range(2):
                    nc.tensor.matmul(ab_ps[:, k * 512:(k + 1) * 512],
                                     ones1_sb[:],
                                     a3_rows[s][:, k * 512:(k + 1) * 512],
                                     start=True, stop=True)
                ab_sb = work.tile([128, Z], D16, tag="ab_sb")
                nc.scalar.copy(ab_sb[:], ab_ps[:])
                for r in range(NT):
                    col = r * SLOTS + s
                    ub = work.tile([128, Z], D16, tag="ub")
                    nc.sync.dma_start(out=ub[:],
                                      in_=u_in[s][r * 128:(r + 1) * 128, :])
                    e0 = work.tile([128, Z], D16, tag="e0")
                    nc.scalar.activation(out=e0[:], in_=ub[:], func=Exp,
                                         scale=nbeta_sb[:, s:s + 1])
                    nc.vector.tensor_tensor_reduce(
                        out=pn[s][:, r * Z:(r + 1) * Z], in0=e0[:], in1=ab_sb[:],
                        scale=1.0, scalar=0.0, op0=mult, op1=add,
                        accum_out=rowsum_sb[:, col:col + 1])
                    scr = work.tile([128, Z], D16, tag="scr")
                    nc.vector.tensor_tensor_reduce(
                        out=scr[:], in0=pn[s][:, r * Z:(r + 1) * Z], in1=ub[:],
                        scale=1.0, scalar=0.0, op0=mult, op1=add,
                        accum_out=numu_sb[:, col:col + 1])

            # ---- U_ni = numU / rowsum, ship to DRAM, AllGather
            rec_sb = pers.tile([128, NT * SLOTS], F32, tag="rec")
            nc.vector.reciprocal(rec_sb[:], rowsum_sb[:])
            ucol_sb = pers.tile([128, NT * SLOTS], F32, tag="ucol")
            nc.vector.tensor_mul(ucol_sb[:], numu_sb[:], rec_sb[:])
            u_mine = dram.tile([SLOTS, Z], F32, tag="u_mine")
            for r in range(NT):
                for s in range(SLOTS):
                    col = r * SLOTS + s
                    nc.sync.dma_start(
                        out=u_mine[s:s + 1, r * 128:(r + 1) * 128],
                        in_=ucol_sb[:, col:col + 1])
            ug = dram.tile([M, Z], F32, tag="ug")
            nc.gpsimd.collective_compute(
                "AllGather", mybir.AluOpType.bypass, replica_groups=groups,
                ins=[u_mine[:].opt()], outs=[ug[:].opt()])

            # ---- middle section (core's 3 m rows, packed [72, Z])
            u_sb = pers.tile([M, Z], F32, tag="u_sb")
            nc.sync.dma_start(out=u_sb[:], in_=ug[:])
            urep_ps = psum.tile([72, Z], F32, tag="mid_ps")
            for k in range(2):
                nc.tensor.matmul(urep_ps[:, k * 512:(k + 1) * 512], L_u_sb[:],
                                 u_sb[:, k * 512:(k + 1) * 512],
                                 start=True, stop=True)
            expu_sb = pers.tile([72, Z], F32, tag="expu")
            nc.scalar.activation(out=expu_sb[:], in_=urep_ps[:], func=Exp,
                                 scale=ndelta72_sb[:, 0:1])
            a_sb = pers.tile([72, Z], F32, tag="a_m")
            nc.scalar.activation(out=a_sb[:], in_=expu_sb[:], func=Ident,
                                 bias=demin72_sb[:, 0:1],
                                 scale=span72_sb[:, 0:1])
            au_sb = pers.tile([72, Z], F32, tag="au")
            nc.vector.tensor_mul(au_sb[:], a_sb[:], urep_ps[:])
            expe_sb = pers.tile([72, Z], F32, tag="expe")
            nc.scalar.activation(out=expe_sb[:], in_=au_sb[:], func=Exp,
                                 scale=somg72_sb[:, 0:1])
            em_sb = pers.tile([72, Z], F32, tag="em")
            nc.vector.tensor_mul(em_sb[:], expe_sb[:], attr72_sb[:])
            zd_ps = psum.tile([SLOTS, Z], F32, tag="mid_ps")
            for k in range(2):
                nc.tensor.matmul(zd_ps[:, k * 512:(k + 1) * 512], L_zd_sb[:],
                                 em_sb[:, k * 512:(k + 1) * 512],
                                 start=True, stop=True)
            zd2_sb = pers.tile([SLOTS, Z], F32, tag="zd2")
            nc.vector.tensor_scalar_add(zd2_sb[:], zd_ps[:], ram3_sb[:, 0:1])
            zinv_sb = pers.tile([SLOTS, Z], F32, tag="zinv")
            nc.vector.reciprocal(zinv_sb[:], zd2_sb[:])
            zrep_ps = psum.tile([72, Z], F32, tag="mid_ps")
            for k in range(2):
                nc.tensor.matmul(zrep_ps[:, k * 512:(k + 1) * 512], L_b3_sb[:],
                                 zinv_sb[:, k * 512:(k + 1) * 512],
                                 start=True, stop=True)
            se_sb = pers.tile([72, Z], F32, tag="se")
            nc.vector.tensor_mul(se_sb[:], em_sb[:], zrep_ps[:])
            se2_sb = pers.tile([72, Z], F32, tag="se2")
            nc.vector.tensor_scalar_add(se2_sb[:], se_sb[:], invcm72_sb[:, 0:1])
            ax_sb = pers.tile([72, Z], F32, tag="ax")
            nc.vector.tensor_mul(ax_sb[:], a_sb[:], x72_sb[:])
            g_sb = pers.tile([72, Z], F32, tag="g_m")
            nc.vector.tensor_mul(g_sb[:], se2_sb[:], ax_sb[:])
            dp_ps = psum.tile([M, Z], F32, tag="mid_ps")
            for k in range(2):
                nc.tensor.matmul(dp_ps[:, k * 512:(k + 1) * 512], L_d_sb[:],
                                 g_sb[:, k * 512:(k + 1) * 512],
                                 start=True, stop=True)
            dp_sb = pers.tile([M, Z], F32, tag="dp_sb")
            nc.scalar.copy(dp_sb[:], dp_ps[:])
            dpart = dram.tile([M, Z], F32, tag="dpart")
            nc.sync.dma_start(out=dpart[:], in_=dp_sb[:])
            dred = dram.tile([SLOTS, Z], F32, tag="dred")
            nc.gpsimd.collective_compute(
                "ReduceScatter", add, replica_groups=groups,
                ins=[dpart[:].opt()], outs=[dred[:].opt()])

            # ---- pass 2: w = (D + exog)/rowsum; X_pred = w^T @ Pnum
            dr_sb = pers.tile([SLOTS, Z], F32, tag="dr")
            nc.sync.dma_start(out=dr_sb[:], in_=dred[:])
            dfix_sb = pers.tile([SLOTS, Z], F32, tag="dfix")
            nc.vector.tensor_add(dfix_sb[:], dr_sb[:], exog3_sb[:])
            dt_ps = psum.tile([128, NT * SLOTS], F32, tag="dt_ps")
            for r in range(NT):
                nc.tensor.transpose(dt_ps[:, r * SLOTS:(r + 1) * SLOTS],
                                    dfix_sb[:, r * 128:(r + 1) * 128],
                                    I3_sb[:])
            w_sb = pers.tile([128, NT * SLOTS], D16, tag="w_sb")
            nc.vector.tensor_mul(w_sb[:], dt_ps[:], rec_sb[:])
            for s in range(SLOTS):
                xp_ps = psum.tile([1, Z], F32, tag="xp_ps")
                for r in range(NT):
                    col = r * SLOTS + s
                    for k in range(2):
                        nc.tensor.matmul(
                            xp_ps[:, k * 512:(k + 1) * 512],
                            w_sb[:, col:col + 1],
                            pn[s][:, r * Z + k * 512: r * Z + (k + 1) * 512],
                            start=(r == 0), stop=(r == NT - 1))
                xp_row = work.tile([1, Z], F32, tag="xp_row")
                nc.scalar.copy(xp_row[:], xp_ps[:])
                nc.sync.dma_start(out=xp_t[s:s + 1, :], in_=xp_row[:])

    return nc


# ---------------------------------------------------------------- host finish
def _host_mse(xp_rows, inputs):
    X_target = np.float32(1) * np.asarray(inputs['X_target'], np.float32)
    X_pred = np.concatenate(xp_rows, axis=0).astype(np.float32)
    return np.float32(np.mean((X_pred - X_target) ** 2, dtype=np.float64))


def _numpy_fallback(inputs):
    """Bit-similar numpy version of the device pipeline."""
    f32 = np.float32
    in_maps = _bake(inputs)
    xp_rows = []
    rowsum_all = np.empty((M, Z), f32)
    pn_all = {}
    U_ni = np.empty((M, Z), f32)
    for c in range(NC):
        im = in_maps[c]
        for s in range(SLOTS):
            n = 3 * c + s
            ub = f32(im[f"u{s}"])
            e0 = f32(f32(np.exp(im["nbeta"][0, s] * ub)).astype(BF16))
            p = f32((e0 * f32(im["a3"][s])[None, :]).astype(BF16))
            pn_all[n] = p
            rowsum_all[n] = p.sum(axis=1)
            scr = f32((p * ub).astype(BF16))
            U_ni[n] = scr.sum(axis=1) / rowsum_all[n]
    # middle in f32 (full, equivalent to sharded+RS)
    demin = f32(inputs['demin']); demax = f32(inputs['demax'])
    delta = f32(inputs['delta']); omega = f32(inputs['omega'])
    sigma = f32(inputs['sigma']); Kn = f32(inputs['Kn'])
    attractor = f32(inputs['attractor'])
    X_total = f32(inputs['X_0']) + f32(inputs['exog_prod'])
    cmask = (Kn > 0).astype(f32)
    expU = np.exp(-delta[:, :, None] * U_ni[None])
    a = demin[:, :, None] + (demax - demin)[:, :, None] * expU
    expE = np.exp((-sigma[:, None] * omega)[:, :, None] * (a * U_ni[None]))
    em = expE * (cmask[:, :, None] * attractor[:, None, :])
    Zd = em.sum(axis=1) + (cmask.sum(axis=1) == 0).astype(f32)[:, None]
    S = em / Zd[:, None, :] + (1.0 - cmask)[:, :, None]
    D = f32(inputs['exog_demand']) + (a * S * X_total[:, None, :]).sum(axis=0)
    for n in range(M):
        w = f32((D[n] / rowsum_all[n]).astype(BF16))
        xp_rows.append((w[:, None] * pn_all[n]).sum(axis=0)[None, :])
    return _host_mse(xp_rows, inputs)


def _run_device(in_maps, trace=False, trace_cores=None):
    import sys
    if '/opt/trn_rl_repo' not in sys.path:
        sys.path.insert(0, '/opt/trn_rl_repo')
    from concourse.bass_utils import run_bass_kernel_spmd
    nc = _build_program()
    res = run_bass_kernel_spmd(nc, in_maps, list(range(NC)), trace=trace,
                               trace_cores=trace_cores)
    return res


def kernel(**inputs):
    inputs = {k: np.asarray(v) for k, v in inputs.items()}
    try:
        in_maps = _bake(inputs)
        res = _run_device(in_maps)
        xp_rows = [res.results[c]["xp"] for c in range(NC)]
        return _host_mse(xp_rows, inputs)
    except Exception:
        import traceback
        traceback.print_exc()
        return _numpy_fallback(inputs)
